# revision 20
# baseline (speedup 1.0000x reference)
"""AutoCorrelation (Autoformer) Trainium2 kernel, 8-core data-parallel over batch.

Algorithm per core (one batch b):
  mean_value[b, tau] = (1/(H*E)) sum_c circular-crosscorr(q[:,c], k[:,c])[tau]
computed via a 16-subsequence DFT-128 decomposition (t = 16u + r):
  - stage A: per (r, c): DFT_128 of subsequence -> packed spectrum (re 0..64 | im 1..63)
    as fp32 matmuls with the data as the stationary operand (output partitions = c).
  - stage P: cross-spectra P[a,b,f] = sum_c Qf[c,a,f] * conj-parts(Kf[c,b,f]) as
    tiny [K=128,M=16,N=16] matmuls accumulated over 4 c-chunks.
  - diagonal sums of the 16x16 blocks (partition-fold with shifted APs),
    twiddle combine, then one IDFT-128 matmul -> mean_value[b] (exact fp32).
  - AllReduce(sum) over the 8 cores -> scores; on-device top-7 (vector.max/max_index),
    softmax over the 7 gathered mean_value entries -> weights.
  - output: out[t,c] = sum_i w_i * v[(t+delta_i) % L, c] via transposed V in SBUF
    (PE transposes) + 7 dynamically-offset (register ds) fused multiply-adds on DVE,
    then PE transpose back.
"""

import os
import sys
import numpy as np

for p in ("/opt/trn_rl_repo",):
    if p not in sys.path and os.path.isdir(p):
        sys.path.insert(0, p)

import concourse.bass as bass
import concourse.bacc as bacc
import concourse.tile as tile
import concourse.mybir as mybir
from concourse import bass_utils
import concourse.bass_isa as bass_isa

F32 = mybir.dt.float32
U32 = mybir.dt.uint32
AL = mybir.AluOpType

B, L, H, E = 8, 2048, 8, 64
C = H * E            # 512
U, R = 128, 16       # L = R * U ; t = 16*u + r
NF = 65              # rfft freqs of DFT-128 we keep (0..64)
NCORES = 8
TOPK = 7


def _consts():
    u = np.arange(U)
    # stage-A DFT matrix, packed: cols 0..64 = cos, cols 65..127 = -sin (f=1..63)
    wdft = np.zeros((U, 128), dtype=np.float32)
    f = np.arange(NF)
    wdft[:, :NF] = np.cos(2 * np.pi * np.outer(u, f) / U)
    f2 = np.arange(1, 64)
    wdft[:, NF:] = -np.sin(2 * np.pi * np.outer(u, f2) / U)

    # IDFT matrix on packed spectrum -> mean_value (incl 2x Hermitian weight and 1/(U*C))
    widft = np.zeros((128, U), dtype=np.float32)
    v = np.arange(U)
    scale = np.ones(NF); scale[1:64] = 2.0
    norm = 1.0 / (U * C)
    widft[:NF, :] = (scale[:, None] * np.cos(2 * np.pi * np.outer(f, v) / U)) * norm
    widft[NF:, :] = (-2.0 * np.sin(2 * np.pi * np.outer(f2, v) / U)) * norm

    # twiddle vectors for the lo-diagonal combine; SP rows 0..64 are S_re(f),
    # rows 65..127 are S_im(f'), f'=1..63.
    twv = np.zeros((128, 2), dtype=np.float32)
    twv[:NF, 0] = np.cos(2 * np.pi * f / U)
    twv[:NF, 1] = -np.sin(2 * np.pi * f / U)
    twv[NF:, 0] = np.cos(2 * np.pi * f2 / U)
    twv[NF:, 1] = np.sin(2 * np.pi * f2 / U)

    ident = np.eye(128, dtype=np.float32)
    return wdft, widft, twv, ident


def build_kernel(nc):
    q_ext = nc.dram_tensor("q", [L, C], F32, kind="ExternalInput")
    k_ext = nc.dram_tensor("k", [L, C], F32, kind="ExternalInput")
    v_ext = nc.dram_tensor("v", [L, C], F32, kind="ExternalInput")
    wdft_ext = nc.dram_tensor("wdft", [U, 128], F32, kind="ExternalInput")
    widft_ext = nc.dram_tensor("widft", [128, U], F32, kind="ExternalInput")
    twv_ext = nc.dram_tensor("twv", [128, 2], F32, kind="ExternalInput")
    ident_ext = nc.dram_tensor("ident", [128, 128], F32, kind="ExternalInput")
    out_ext = nc.dram_tensor("out", [L, C], F32, kind="ExternalOutput")
    dbg_mv = nc.dram_tensor("dbg_mv", [L], F32, kind="ExternalOutput")
    dbg_sc = nc.dram_tensor("dbg_sc", [L], F32, kind="ExternalOutput")
    dbg_idx = nc.dram_tensor("dbg_idx", [8], U32, kind="ExternalOutput")
    dbg_wv = nc.dram_tensor("dbg_wv", [8], F32, kind="ExternalOutput")
    dbg_vt = nc.dram_tensor("dbg_vt", [128, 32], F32, kind="ExternalOutput")
    dbg_pp = nc.dram_tensor("dbg_pp", [16, 2 * NF * 16], F32, kind="ExternalOutput")
    dbg_dg = nc.dram_tensor("dbg_dg", [16, 2 * NF * 48], F32, kind="ExternalOutput")
    dbg_sp = nc.dram_tensor("dbg_sp", [128, 64], F32, kind="ExternalOutput")

    NCH = C // 128  # 4 channel chunks

    with tile.TileContext(nc) as tc:
        with (
            tc.tile_pool(name="const", bufs=1) as constp,
            tc.tile_pool(name="spec", bufs=1) as specp,
            tc.tile_pool(name="stage", bufs=2) as stagep,
            tc.tile_pool(name="small", bufs=1) as smallp,
            tc.tile_pool(name="ot", bufs=2) as otp,
            tc.tile_pool(name="psA", bufs=2, space="PSUM") as psA,
            tc.tile_pool(name="psP", bufs=2, space="PSUM") as psP,
            tc.tile_pool(name="psT", bufs=2, space="PSUM") as psT,
            tc.tile_pool(name="dram", bufs=1, space="DRAM") as dramp,
        ):
            # ---- constants ----
            wdft_sb = constp.tile([U, 128], F32, tag="wdft")
            widft_sb = constp.tile([128, U], F32, tag="widft")
            twv_sb = constp.tile([128, 2], F32, tag="twv")
            ident_sb = constp.tile([128, 128], F32, tag="ident")
            nc.sync.dma_start(wdft_sb[:], wdft_ext.ap())
            nc.sync.dma_start(widft_sb[:], widft_ext.ap())
            nc.sync.dma_start(twv_sb[:], twv_ext.ap())
            nc.sync.dma_start(ident_sb[:], ident_ext.ap())

            # ---- stage A: subsequence spectra ----
            # spectra tiles: QS[ch] [c=128, r*128 + fpack]
            QS = [specp.tile([128, R * 128], F32, tag=f"qs{ch}", name=f"qs{ch}") for ch in range(NCH)]
            KS = [specp.tile([128, R * 128], F32, tag=f"ks{ch}", name=f"ks{ch}") for ch in range(NCH)]

            for side, (ext, SS) in enumerate(((q_ext, QS), (k_ext, KS))):
                # view [L, C] as [ch, u, (r c)]
                src = ext.ap().rearrange(
                    "(u r) (g c) -> g u r c", r=R, c=128
                )
                for ch in range(NCH):
                    x = stagep.tile([128, R * 128], F32, tag="xstage")
                    nc.sync.dma_start(x[:], src[ch])
                    for r in range(R):
                        ps = psA.tile([128, 128], F32, tag="psa")
                        nc.tensor.matmul(
                            ps[:],
                            x[:, r * 128:(r + 1) * 128],
                            wdft_sb[:],
                            start=True, stop=True,
                        )
                        # K side stores r-blocks reversed so the later
                        # cross-spectrum diagonals become ascending in x.
                        rs = r if side == 0 else (R - 1 - r)
                        nc.any.tensor_copy(SS[ch][:, rs * 128:(rs + 1) * 128], ps[:])

            # ---- stage P: cross spectra, 16x16 per f ----
            # PP planes [16, f*16 + b]
            PPre = specp.tile([16, NF * 16], F32, tag="ppre")
            PPim = specp.tile([16, NF * 16], F32, tag="ppim")

            QSr = [QS[ch].rearrange("c (r fp) -> c fp r", fp=128) for ch in range(NCH)]
            KSr = [KS[ch].rearrange("c (r fp) -> c fp r", fp=128) for ch in range(NCH)]

            fgroups = [list(range(g * 8, min(g * 8 + 8, NF))) for g in range((NF + 7) // 8)]
            for fg in fgroups:
                pg = psP.tile([16, 8 * 64], F32, tag="psp")
                for fi, f in enumerate(fg):
                    off = fi * 64
                    prods = [(0, f, f)]
                    if 0 < f < 64:
                        prods += [(16, f, 64 + f), (32, 64 + f, f),
                                  (48, 64 + f, 64 + f)]
                    # product-major: close each 4-chunk PSUM accumulation
                    # group before opening the next one in the same bank.
                    for po, qf, kf in prods:
                        for ch in range(NCH):
                            nc.tensor.matmul(
                                pg[:, off + po:off + po + 16],
                                QSr[ch][:, qf, :], KSr[ch][:, kf, :],
                                start=(ch == 0), stop=(ch == NCH - 1))
                # combine: P_re = A + D ; P_im = C' - B
                # (DVE has a single PSUM read port: stage to SBUF first)
                pgs = stagep.tile([16, 8 * 64], F32, tag="pgs", name="pgs")
                for fi, f in enumerate(fg):
                    n = 16 if (f == 0 or f == 64) else 64
                    nc.any.tensor_copy(
                        pgs[:, fi * 64:fi * 64 + n], pg[:, fi * 64:fi * 64 + n])
                pgv = pgs.rearrange("p (f x) -> p f x", x=64)
                for fi, f in enumerate(fg):
                    dst = slice(f * 16, f * 16 + 16)
                    if 0 < f < 64:
                        nc.vector.tensor_add(
                            PPre[:, dst], pgv[:, fi, 0:16], pgv[:, fi, 48:64])
                        nc.vector.tensor_sub(
                            PPim[:, dst], pgv[:, fi, 32:48], pgv[:, fi, 16:32])
                    else:
                        nc.any.tensor_copy(PPre[:, dst], pgv[:, fi, 0:16])
                        nc.vector.memset(PPim[:, dst], 0.0)

            # ---- diagonal sums of the 16x16 blocks ----
            # skew rows (row a shifted right by 16-a within each 48-block) then
            # all-reduce across partitions; d_s(f) lands at x = 16 - s of row 0.
            SKre = specp.tile([16, NF * 48], F32, tag="skre")
            SKim = specp.tile([16, NF * 48], F32, tag="skim")
            DGre = specp.tile([16, NF * 48], F32, tag="dgre")
            DGim = specp.tile([16, NF * 48], F32, tag="dgim")
            for SK, PP, DG in ((SKre, PPre, DGre), (SKim, PPim, DGim)):
                nc.vector.memset(SK[:], 0.0)
                skv = SK.rearrange("p (f x) -> p f x", x=48)
                ppv = PP.rearrange("p (f b) -> p f b", b=16)
                for a in range(16):
                    nc.sync.dma_start(
                        skv[a:a + 1, :, 1 + a:17 + a], ppv[a:a + 1, :, :])
                nc.gpsimd.partition_all_reduce(
                    DG[:], SK[:], channels=16, reduce_op=bass_isa.ReduceOp.add)

            # ---- build packed spectrum SP [128, 16] and IDFT ----
            HI = smallp.tile([128, R], F32, tag="hi")
            LO1 = smallp.tile([128, R], F32, tag="lo1")
            LO2 = smallp.tile([128, R], F32, tag="lo2")
            # hi: d_{s=w} at x=16-w -> negative step from x=16 ; lo: d_{s=w-16} at 32-w
            tfre_r = DGre.rearrange("p (f x) -> p f x", x=48)
            tfim_r = DGim.rearrange("p (f x) -> p f x", x=48)

            # DG row 0 -> DRAM (contiguous), then strided DRAM->SBUF scatters.
            dgrow_re = dramp.tile([NF * 48], F32, tag="dgrowre", name="dgrow_re")
            dgrow_im = dramp.tile([NF * 48], F32, tag="dgrowim", name="dgrow_im")
            nc.sync.dma_start(dgrow_re.rearrange("(o x) -> o x", o=1), DGre[0:1, :])
            nc.sync.dma_start(dgrow_im.rearrange("(o x) -> o x", o=1), DGim[0:1, :])
            rrv = dgrow_re.rearrange("(f x) -> f x", x=48)
            irv = dgrow_im.rearrange("(f x) -> f x", x=48)
            # hi[w] = d_{s=w} at x=16+w ; lo[w] = d_{s=w-16} at x=w
            nc.sync.dma_start(HI[0:NF, :], rrv[0:NF, 16:32])
            nc.sync.dma_start(HI[NF:128, :], irv[1:64, 16:32])
            nc.sync.dma_start(LO1[0:NF, :], rrv[0:NF, 0:16])
            nc.sync.dma_start(LO1[NF:128, :], irv[1:64, 0:16])
            nc.sync.dma_start(LO2[0:NF, :], irv[0:NF, 0:16])
            nc.sync.dma_start(LO2[NF:128, :], rrv[1:64, 0:16])

            SP = smallp.tile([128, R], F32, tag="sp")
            t1 = smallp.tile([128, R], F32, tag="t1")
            nc.vector.tensor_scalar(t1[:], LO1[:], twv_sb[:, 0:1], None, AL.mult)
            nc.vector.tensor_add(SP[:], t1[:], HI[:])
            nc.vector.tensor_scalar(t1[:], LO2[:], twv_sb[:, 1:2], None, AL.mult)
            nc.vector.tensor_add(SP[:], SP[:], t1[:])

            nc.sync.dma_start(dbg_sp[:, 0:16], SP[:])
            nc.sync.dma_start(dbg_sp[:, 16:32], HI[:])
            nc.sync.dma_start(dbg_sp[:, 32:48], LO1[:])
            nc.sync.dma_start(dbg_sp[:, 48:64], LO2[:])
            ps_mv = psA.tile([128, R], F32, tag="psmv", bufs=1)
            nc.tensor.matmul(ps_mv[:], widft_sb[:], SP[:], start=True, stop=True)
            mv_sb = smallp.tile([128, R], F32, tag="mv")
            nc.any.tensor_copy(mv_sb[:], ps_mv[:])

            # ---- all-reduce scores over batch ----
            mv_dram = dramp.tile([L], F32, tag="mvd")
            sc_dram = dramp.tile([L], F32, tag="scd")
            nc.gpsimd.dma_start(mv_dram.rearrange("(p w) -> p w", w=R), mv_sb[:])
            nc.gpsimd.collective_compute(
                "AllReduce",
                AL.add,
                replica_groups=[list(range(NCORES))],
                ins=[mv_dram.opt()],
                outs=[sc_dram.opt()],
            )

            # ---- top-7 + softmax weights ----
            sc_sb = smallp.tile([1, L], F32, tag="scsb")
            mvl_sb = smallp.tile([1, L], F32, tag="mvl")
            nc.gpsimd.dma_start(sc_sb[:], sc_dram.rearrange("(o l) -> o l", o=1))
            nc.gpsimd.dma_start(mvl_sb[:], mv_dram.rearrange("(o l) -> o l", o=1))
            mx8 = smallp.tile([1, 8], F32, tag="mx8")
            idx8 = smallp.tile([1, 8], U32, tag="idx8")
            nc.vector.max(mx8[:], sc_sb[:])
            nc.vector.max_index(idx8[:], mx8[:], sc_sb[:])

            deltas = []
            for i in range(TOPK):
                deltas.append(
                    nc.values_load(idx8[0:1, i:i + 1], min_val=0, max_val=L - 1,
                                   skip_runtime_bounds_check=True)
                )

            wv = smallp.tile([1, 8], F32, tag="wv")
            nc.vector.memset(wv[:], 0.0)
            for i in range(TOPK):
                nc.any.tensor_copy(
                    wv[0:1, i:i + 1], mvl_sb[0:1, bass.ds(deltas[i], 1)])
            wmax = smallp.tile([1, 1], F32, tag="wmax")
            nc.vector.reduce_max(wmax[:], wv[0:1, 0:TOPK], axis=mybir.AxisListType.X)
            nc.vector.tensor_scalar(
                wv[0:1, 0:TOPK], wv[0:1, 0:TOPK], wmax[:], None, AL.subtract)
            nc.scalar.activation(
                wv[0:1, 0:TOPK], wv[0:1, 0:TOPK], mybir.ActivationFunctionType.Exp)
            wsum = smallp.tile([1, 1], F32, tag="wsum")
            nc.vector.reduce_sum(wsum[:], wv[0:1, 0:TOPK], axis=mybir.AxisListType.X)
            wrec = smallp.tile([1, 1], F32, tag="wrec")
            nc.vector.reciprocal(wrec[:], wsum[:])
            nc.vector.tensor_scalar(
                wv[0:1, 0:TOPK], wv[0:1, 0:TOPK], wrec[:], None, AL.mult)
            wb = smallp.tile([128, 8], F32, tag="wb")
            nc.gpsimd.partition_broadcast(wb[:, 0:8], wv[0:1, 0:8])

            nc.sync.dma_start(dbg_mv.ap().rearrange("(p w) -> p w", w=R), mv_sb[:])
            nc.sync.dma_start(dbg_sc.ap().rearrange("(o l) -> o l", o=1), sc_sb[:])
            nc.sync.dma_start(dbg_idx.ap().rearrange("(o l) -> o l", o=1), idx8[:])
            nc.sync.dma_start(dbg_wv.ap().rearrange("(o l) -> o l", o=1), wv[:])

            # ---- V transpose into [c, t] doubled ----
            VT = [specp.tile([128, 2 * L], F32, tag=f"ks{ch}", name=f"vt{ch}") for ch in range(NCH)]
            vsrc = v_ext.ap().rearrange("(j p) (g c) -> g j p c", p=128, c=128)
            for ch in range(NCH):
                for j in range(L // 128):
                    vtile = stagep.tile([128, 128], F32, tag="vstage")
                    nc.sync.dma_start(vtile[:], vsrc[ch, j])
                    pst = psT.tile([128, 128], F32, tag="pst")
                    nc.tensor.transpose(pst[:], vtile[:], ident_sb[:])
                    nc.any.tensor_copy(VT[ch][:, j * 128:(j + 1) * 128], pst[:])
                nc.any.tensor_copy(VT[ch][:, L:2 * L], VT[ch][:, 0:L])

            # ---- 7-tap weighted shifted sum on DVE ----
            # reference: rolled[l] = v[(l - delta) % L] -> doubled-V offset L - delta
            offs = [L - d for d in deltas]
            ACC = [specp.tile([128, L], F32, tag=f"qs{ch}", name=f"acc{ch}") for ch in range(NCH)]
            for ch in range(NCH):
                nc.vector.tensor_scalar(
                    ACC[ch][:], VT[ch][:, bass.ds(offs[0], L)],
                    wb[:, 0:1], None, AL.mult)
                for i in range(1, TOPK):
                    nc.vector.scalar_tensor_tensor(
                        ACC[ch][:],
                        VT[ch][:, bass.ds(offs[i], L)],
                        wb[:, i:i + 1],
                        ACC[ch][:],
                        op0=AL.mult, op1=AL.add)

            nc.sync.dma_start(dbg_vt[:, 0:16], VT[0][:, 0:16])
            nc.sync.dma_start(dbg_vt[:, 16:32], ACC[0][:, 0:16])

            # ---- transpose back and store ----
            for j in range(L // 128):
                ot = otp.tile([128, C], F32, tag="ot")
                for ch in range(NCH):
                    pst = psT.tile([128, 128], F32, tag="pst")
                    nc.tensor.transpose(
                        pst[:], ACC[ch][:, j * 128:(j + 1) * 128], ident_sb[:])
                    nc.any.tensor_copy(ot[:, ch * 128:(ch + 1) * 128], pst[:])
                nc.sync.dma_start(
                    out_ext.ap().rearrange("(j p) c -> j p c", p=128)[j], ot[:])

    return nc


_NC_CACHE = {}


def _get_nc():
    if "nc" not in _NC_CACHE:
        nc = bacc.Bacc(
            "TRN2", target_bir_lowering=False, debug=False, num_devices=NCORES)
        build_kernel(nc)
        nc.compile()
        _NC_CACHE["nc"] = nc
    return _NC_CACHE["nc"]


def _in_maps(queries, keys, values):
    wdft, widft, twv, ident = _consts()
    maps = []
    for b in range(B):
        maps.append({
            "q": np.ascontiguousarray(queries[b].reshape(L, C), dtype=np.float32),
            "k": np.ascontiguousarray(keys[b].reshape(L, C), dtype=np.float32),
            "v": np.ascontiguousarray(values[b].reshape(L, C), dtype=np.float32),
            "wdft": wdft, "widft": widft, "twv": twv, "ident": ident,
        })
    return maps


def run(queries, keys, values, trace=False):
    nc = _get_nc()
    res = bass_utils.run_bass_kernel_spmd(
        nc, _in_maps(queries, keys, values),
        core_ids=list(range(NCORES)), trace=trace)
    outs = [res.results[b]["out"].reshape(L, H, E) for b in range(B)]
    return np.stack(outs, axis=0), res


def kernel(queries, keys, values, attn_mask=None):
    out, _ = run(np.asarray(queries), np.asarray(keys), np.asarray(values))
    return out.astype(np.float32)
